# revision 38
# baseline (speedup 1.0000x reference)
"""Bass/Tile TRN2 kernel for LunarAttention (RoPE attention, B=2, S=2048, E=2048, H=16).

Sharding: 8 cores = 2 batches x 4 head-groups (4 heads / 512 dims each).

v2: all four projections (Q/K/V/O) run as 3-term fp8e4m3 DoubleRow matmuls
(hi*hi pair-chunks + cross terms, dropping lo*lo), which the PE executes at
0.5 cycles/row -> 0.75x the bf16 cycle count per projection. Operands are
split host-side into (hi, lo) fp8 pairs after scaling into e4m3's normal
range (hs x8, W x64, Wo x64; ao is scaled x32 by folding 32 into the V
eviction), with descales folded into existing scale-capable evictions.
Scores stay f32r and attn@V stays bf16 (fp8 noise there fails the 2e-2
rel-err gate; see err_study.py).

Per core (batch b, head-group g):
  Phase A: kT = rope((Wk_g hs^T)/512 + bk) in [d, s]; vS = (hs Wv_g^T)/16
           (x32 net after /512) in [s, d]. (v bias handled host-side.)
  Phase B per 512-wide q chunk: q projection + rope, then per head
           scoresT[kv,q] -> exp (scalar engine) -> attn@v accumulation on PE.
           Softmax denominators via DVE/Pool accumulation of exp tiles +
           partition_all_reduce. ao = po*recb split into (hi, lo) fp8 on DVE.
           Out-projection of the previous q chunk and Q-projection of the
           next are interleaved into the attention loop (3 filler slots per
           kv iteration: 96 oproj + 96 qproj matmuls per chunk).
dtypes: proj operands fp8e4m3 (DoubleRow), attention math fp32/f32r/bf16,
PSUM fp32, out bf16 (x2048, host descales).
"""

import math
import numpy as np

B, S, E, H, D = 2, 2048, 2048, 16, 128
G = 4            # head groups (cores per batch)
HPG = 4          # heads per group
F = HPG * D      # 512 dims per group
P = 128
EC = E // P      # 16 contraction chunks for projections
SB = 512         # phase-A s-block width
NSB = S // SB    # 4
QW = 512         # phase-B q chunk width
NQC = S // QW    # 4
KVC = S // P     # 16 kv chunks
NEC = 4          # out-proj e chunks of 512
SCALE = 1.0 / math.sqrt(D)

HS_SC = 8.0      # host scale on hs before fp8 split
W_SC = 64.0      # host scale on Wq/Wk/Wv/Wo before fp8 split
AO_SC = 32.0     # ao scale (folded into V eviction)
PROJ_DESC = 1.0 / (HS_SC * W_SC)          # 1/512: Q/K psum descale
V_DESC = AO_SC / (HS_SC * W_SC)           # 1/16: V psum eviction scale
O_DESC = 1.0 / (AO_SC * W_SC)             # 1/2048: out-proj descale (host)
NDR = 0          # number of heads (the last NDR) using fp8-DR attn@V
H0DR = HPG - NDR  # first DR head
EXP_SHIFT = -2.0  # exp bias for fp8 ex heads (keeps e^s under e4m3 max)

_NC_CACHE = {}


def _build_nc():
    import concourse.bass as bass
    import concourse.mybir as mybir
    import concourse.tile as tile
    from concourse import bacc
    from concourse import bass_isa
    from contextlib import ExitStack

    f32 = mybir.dt.float32
    f32r = mybir.dt.float32r
    bf16 = mybir.dt.bfloat16
    fp8 = mybir.dt.float8e4
    AF = mybir.ActivationFunctionType
    PM = mybir.MatmulPerfMode.DoubleRow

    nc = bacc.Bacc("TRN2", target_bir_lowering=False)

    # hi/lo fp8 pairs, host-packed to match the SBUF tile layouts exactly so
    # every DMA collapses to <=3 dims. hs t-dim = (hi, lo), weights (lo, hi)
    # so a [:, :, ec, :] cross-slice pairs hs_hi*w_lo + hs_lo*w_hi.
    hs_d = nc.dram_tensor("hsX", [NSB, P, 2 * EC * SB], fp8, kind="ExternalInput")
    wq_d = nc.dram_tensor("wqX", [P, 2 * EC * F], fp8, kind="ExternalInput")
    wk_d = nc.dram_tensor("wkX", [P, 2 * EC * F], fp8, kind="ExternalInput")
    wv_d = nc.dram_tensor("wvX", [P, 2 * EC * F], fp8, kind="ExternalInput")
    wo_d = nc.dram_tensor("woX", [P, 2 * HPG * E], fp8, kind="ExternalInput")
    bq_d = nc.dram_tensor("bq2", [P, HPG], f32, kind="ExternalInput")
    bk_d = nc.dram_tensor("bk2", [P, HPG], f32, kind="ExternalInput")
    cos_d = nc.dram_tensor("cosT", [P, S], bf16, kind="ExternalInput")
    sin_d = nc.dram_tensor("sinR", [P, S], bf16, kind="ExternalInput")
    out_d = nc.dram_tensor("out", [S, E], bf16, kind="ExternalOutput")

    def hs_blk(sb):
        return hs_d[sb, :, :].rearrange("p (t c s) -> p t c s", t=2, c=EC)

    wq_r = wq_d[:, :].rearrange("p (t c f) -> p t c f", t=2, c=EC)
    wk_r = wk_d[:, :].rearrange("p (t c f) -> p t c f", t=2, c=EC)
    wv_r = wv_d[:, :].rearrange("p (t c f) -> p t c f", t=2, c=EC)
    wo_r = wo_d[:, :].rearrange("p (t c e) -> p t c e", t=2, c=HPG)

    with tile.TileContext(nc) as tc, ExitStack() as ctx:
        const = ctx.enter_context(tc.tile_pool(name="const", bufs=1))
        persist = ctx.enter_context(tc.tile_pool(name="persist", bufs=1))
        wpool = ctx.enter_context(tc.tile_pool(name="wpool", bufs=1))

        cos_sb = const.tile([P, S], bf16, tag="cos")
        sin_sb = const.tile([P, S], bf16, tag="sin")
        bq_sb = const.tile([P, HPG], f32, tag="bq")
        bk_sb = const.tile([P, HPG], f32, tag="bk")

        wq_sb = wpool.tile([P, 2, EC, F], fp8, tag="wq")
        wk_sb = wpool.tile([P, 2, EC, F], fp8, tag="wk")
        wv_sb = wpool.tile([P, 2, EC, F], fp8, tag="wv")
        wo_sb = wpool.tile([P, 2, HPG, E], fp8, tag="wo")

        def _load_trig_chunk(sb):
            csl = slice(sb * SB, (sb + 1) * SB)
            nc.scalar.dma_start(out=cos_sb[:, csl], in_=cos_d[:, csl])
            nc.scalar.dma_start(out=sin_sb[:, csl], in_=sin_d[:, csl])

        def _deferred_consts_sb1():
            nc.scalar.dma_start(out=bq_sb[:, :], in_=bq_d[:, :])
            nc.sync.dma_start(out=wq_sb[:, :, :, :], in_=wq_r[:, :, :, :])
            nc.sync.dma_start(out=wo_sb[:, :, :, :], in_=wo_r[:, :, :, :])

        kT = [persist.tile([P, S], bf16, tag=f"kT{h}", name=f"kT{h}")
              for h in range(HPG)]
        vS = [persist.tile([P, F], bf16, tag=f"vS{i}", name=f"vS{i}")
              for i in range(S // P)]
        if NDR:
            vhi = persist.tile([P, KVC, NDR * D], fp8, tag="vhi")
            vlo = persist.tile([P, KVC, NDR * D], fp8, tag="vlo")
            nbias = const.tile([P, 1], f32, tag="nbias")
            nc.gpsimd.memset(nbias[:, :], EXP_SHIFT)

        def proj3t(ps, w_sb, hs_sb, fsl, ssl, lhs_is_w=True):
            """Emit the 24 DoubleRow matmuls of one 3-term projection tile:
            8 hi*hi pair-chunks then 16 cross chunks (hi*lo + lo*hi).
            Generator: yields after each matmul emission (for PE fillers)."""
            for e in range(EC // 2):
                es = slice(2 * e, 2 * e + 2)
                if lhs_is_w:
                    nc.tensor.matmul(ps, w_sb[:, 1, es, fsl], hs_sb[:, 0, es, ssl],
                                     start=(e == 0), stop=False, perf_mode=PM)
                else:
                    nc.tensor.matmul(ps, hs_sb[:, 0, es, ssl], w_sb[:, 1, es, fsl],
                                     start=(e == 0), stop=False, perf_mode=PM)
                yield
            for ec in range(EC):
                if lhs_is_w:
                    nc.tensor.matmul(ps, w_sb[:, :, ec, fsl], hs_sb[:, :, ec, ssl],
                                     start=False, stop=(ec == EC - 1), perf_mode=PM)
                else:
                    nc.tensor.matmul(ps, hs_sb[:, :, ec, ssl], w_sb[:, :, ec, fsl],
                                     start=False, stop=(ec == EC - 1), perf_mode=PM)
                yield

        def rope_halves(raw, dstT, dsl, psl, rp, sb, fc, nm):
            """dstT[:, dsl] = raw*cos[psl] + rotate_half(raw)*sin[psl], on DVE
            via cross-partition reads. sin_sb rows 0:64 hold -sin rolled by 64
            partitions; only outputs are partition-shifted (NCC_IBIR297)."""
            w = dsl.stop - dsl.start
            ta = rp.tile([P, w], f32, tag=f"ta{nm}", name=f"ta_{nm}_{sb}_{fc}")
            tb = rp.tile([P, w], f32, tag=f"tb{nm}", name=f"tb_{nm}_{sb}_{fc}")
            nc.vector.tensor_mul(ta[:, :], raw[:, :], cos_sb[:, psl])
            nc.vector.tensor_mul(tb[0:64, :], raw[64:128, :], sin_sb[64:128, psl])
            nc.vector.tensor_mul(tb[64:128, :], raw[0:64, :], sin_sb[0:64, psl])
            nc.vector.tensor_add(dstT[:, dsl], ta[:, :], tb[:, :])

        # Phase-B hs pool hoisted so chunk 0 can prefetch during phase A.
        hspB = ctx.enter_context(tc.tile_pool(name="hspB", bufs=1))

        def load_hsB(qc):
            hs_q = hspB.tile([P, 2, EC, QW], fp8, tag="hsb", name=f"hsB_{qc}")
            nc.sync.dma_start(out=hs_q[:, :, :, :], in_=hs_blk(qc)[:, :, :, :])
            return hs_q

        # ---- Phase A: K (rope) + V projections ----
        # sb=0 runs the four fc groups round-robin per ec-quarter across four
        # concurrent PSUM banks, so the PE streams behind the quarter-granular
        # wk/hs/wv DMAs instead of idling until the full 6MB lands. One hspA
        # pool (bufs=2) spans all blocks so hs loads double-buffer naturally.
        with tc.tile_pool(name="hspA", bufs=2) as hspA, \
             tc.tile_pool(name="rpk", bufs=2) as rpk:
            hs_sb0 = hspA.tile([P, 2, EC, SB], fp8, tag="hs", name="hsA_0")
            # lead-in: the first hh matmul needs only (wk hi, hs hi) of ec 0-1
            e01 = slice(0, 2)
            nc.sync.dma_start(out=wk_sb[:, 1, e01, :], in_=wk_r[:, 1, e01, :])
            nc.sync.dma_start(out=hs_sb0[:, 0, e01, :], in_=hs_blk(0)[:, 0, e01, :])
            nc.sync.dma_start(out=wk_sb[:, 1, 2:4, :], in_=wk_r[:, 1, 2:4, :])
            nc.sync.dma_start(out=hs_sb0[:, 0, 2:4, :], in_=hs_blk(0)[:, 0, 2:4, :])
            nc.sync.dma_start(out=wk_sb[:, 0, 0:4, :], in_=wk_r[:, 0, 0:4, :])
            nc.sync.dma_start(out=hs_sb0[:, 1, 0:4, :], in_=hs_blk(0)[:, 1, 0:4, :])
            nc.scalar.dma_start(out=bk_sb[:, :], in_=bk_d[:, :])
            _load_trig_chunk(0)
            for cc in range(1, 4):
                ecs = slice(cc * 4, (cc + 1) * 4)
                nc.sync.dma_start(out=wk_sb[:, :, ecs, :], in_=wk_r[:, :, ecs, :])
                nc.sync.dma_start(out=hs_sb0[:, :, ecs, :],
                                  in_=hs_blk(0)[:, :, ecs, :])
            for cc in range(4):
                ecs = slice(cc * 4, (cc + 1) * 4)
                nc.sync.dma_start(out=wv_sb[:, :, ecs, :], in_=wv_r[:, :, ecs, :])
            hs_next = hspA.tile([P, 2, EC, SB], fp8, tag="hs", name="hsA_1")
            nc.sync.dma_start(out=hs_next[:, :, :, :], in_=hs_blk(1)[:, :, :, :])
            with tc.tile_pool(name="ppk0", bufs=1, space="PSUM") as ppk0, \
                 tc.tile_pool(name="ppv0", bufs=4, space="PSUM") as ppv0:
                psK0 = [ppk0.tile([P, SB], f32, tag=f"psk{fc}", name=f"psk_0_{fc}")
                        for fc in range(HPG)]
                for qtr in range(4):
                    for fc in range(HPG):
                        fsl = slice(fc * P, (fc + 1) * P)
                        for e in (2 * qtr, 2 * qtr + 1):
                            es = slice(2 * e, 2 * e + 2)
                            nc.tensor.matmul(psK0[fc][:, :], wk_sb[:, 1, es, fsl],
                                             hs_sb0[:, 0, es, slice(0, SB)],
                                             start=(qtr == 0 and e == 2 * qtr),
                                             stop=False, perf_mode=PM)
                        for ec in range(4 * qtr, 4 * qtr + 4):
                            nc.tensor.matmul(psK0[fc][:, :], wk_sb[:, :, ec, fsl],
                                             hs_sb0[:, :, ec, slice(0, SB)],
                                             start=False,
                                             stop=(qtr == 3 and ec == 4 * qtr + 3),
                                             perf_mode=PM)
                for fc in range(HPG):
                    raw = rpk.tile([P, SB], f32, tag="rawk", name=f"rawk_0_{fc}")
                    nc.scalar.activation(raw[:, :], psK0[fc][:, :], AF.Identity,
                                         bias=bk_sb[:, fc:fc + 1], scale=PROJ_DESC)
                    rope_halves(raw, kT[fc], slice(0, SB), slice(0, SB),
                                rpk, 0, fc, "k")
                for m in range(SB // P):
                    psV = ppv0.tile([P, F], f32, tag="psv", name=f"psv_0_{m}")
                    msl = slice(m * P, (m + 1) * P)
                    for _ in proj3t(psV[:, :], wv_sb, hs_sb0, slice(0, F), msl,
                                    lhs_is_w=False):
                        pass
                    nc.scalar.mul(vS[m][:, :], psV[:, :], V_DESC)
                    if NDR:
                        dsl = slice(H0DR * D, F)
                        nc.vector.tensor_copy(vhi[:, m, :], vS[m][:, dsl])
                        nc.vector.tensor_sub(vlo[:, m, :], vS[m][:, dsl],
                                             vhi[:, m, :])

            with tc.tile_pool(name="ppk", bufs=2, space="PSUM") as ppk, \
                 tc.tile_pool(name="ppv", bufs=2, space="PSUM") as ppv:
                for sb in range(1, NSB):
                    ssl = slice(sb * SB, (sb + 1) * SB)
                    hs_sb = hs_next
                    if sb + 1 < NSB:
                        hs_next = hspA.tile([P, 2, EC, SB], fp8, tag="hs",
                                            name=f"hsA_{sb + 1}")
                        nc.sync.dma_start(out=hs_next[:, :, :, :],
                                          in_=hs_blk(sb + 1)[:, :, :, :])
                        if sb + 1 == NSB - 1:
                            _deferred_consts_sb1()
                    _load_trig_chunk(sb)
                    for fc in range(HPG):
                        psK = ppk.tile([P, SB], f32, tag="psk",
                                       name=f"psk_{sb}_{fc}")
                        fsl = slice(fc * P, (fc + 1) * P)
                        for _ in proj3t(psK[:, :], wk_sb, hs_sb, fsl,
                                        slice(0, SB)):
                            pass
                        raw = rpk.tile([P, SB], f32, tag="rawk",
                                       name=f"rawk_{sb}_{fc}")
                        nc.scalar.activation(raw[:, :], psK[:, :], AF.Identity,
                                             bias=bk_sb[:, fc:fc + 1],
                                             scale=PROJ_DESC)
                        rope_halves(raw, kT[fc], ssl, ssl, rpk, sb, fc, "k")
                    for m in range(SB // P):
                        psV = ppv.tile([P, F], f32, tag="psv",
                                       name=f"psv_{sb}_{m}")
                        msl = slice(m * P, (m + 1) * P)
                        for _ in proj3t(psV[:, :], wv_sb, hs_sb, slice(0, F),
                                        msl, lhs_is_w=False):
                            pass
                        i = sb * (SB // P) + m
                        nc.scalar.mul(vS[i][:, :], psV[:, :], V_DESC)
                        if NDR:
                            dsl = slice(H0DR * D, F)
                            nc.vector.tensor_copy(vhi[:, i, :], vS[i][:, dsl])
                            nc.vector.tensor_sub(vlo[:, i, :], vS[i][:, dsl],
                                                 vhi[:, i, :])

        # ---- Phase B: Q proj + attention + interleaved out-projection ----
        from concourse.bass_isa import ReduceOp
        hsb0 = load_hsB(0)
        with tc.tile_pool(name="qtp", bufs=2) as qtp, \
             tc.tile_pool(name="rpq", bufs=2) as rpq, \
             tc.tile_pool(name="psQ", bufs=2, space="PSUM") as psQ, \
             tc.tile_pool(name="psS", bufs=2, space="PSUM") as psS, \
             tc.tile_pool(name="psO", bufs=2, space="PSUM") as psO, \
             tc.tile_pool(name="psP", bufs=2, space="PSUM") as psP, \
             tc.tile_pool(name="exl", bufs=8) as exl, \
             tc.tile_pool(name="exs", bufs=2) as exs, \
             tc.tile_pool(name="rcp", bufs=2) as rcp, \
             tc.tile_pool(name="aop", bufs=2) as aop, \
             tc.tile_pool(name="otp", bufs=2) as otp:

            def qproj_gen(qc, hs_q, qT):
                """Yield once per Q-projection matmul for chunk qc (24 per
                head-part); bias-add and rope inline as each part completes."""
                qsl = slice(qc * QW, (qc + 1) * QW)
                for fc in range(HPG):
                    ps_q = psQ.tile([P, QW], f32, tag="q", name=f"psq_{qc}_{fc}")
                    fsl = slice(fc * P, (fc + 1) * P)
                    gen = proj3t(ps_q[:, :], wq_sb, hs_q, fsl, slice(0, QW))
                    n = 0
                    for _ in gen:
                        n += 1
                        if n < 24:
                            yield
                    rawq = rpq.tile([P, QW], f32, tag="rawq", name=f"rawq_{qc}_{fc}")
                    nc.scalar.activation(rawq[:, :], ps_q[:, :], AF.Identity,
                                         bias=bq_sb[:, fc:fc + 1], scale=PROJ_DESC)
                    rope_halves(rawq, qT[fc], slice(0, QW), qsl, rpq, qc, fc, "q")
                    yield

            def new_qT(qc):
                return [qtp.tile([P, QW], bf16, tag=f"qT{fc}", name=f"qT_{qc}_{fc}")
                        for fc in range(HPG)]

            def oproj_mms(pp, aos, msl, esl, upto=None):
                """The 6 DoubleRow matmuls of one out-proj tile: 2 hi*hi pairs
                + 4 cross. aos dim1=(hi,lo); wo_sb dim1=(lo,hi).
                upto limits how many to emit (for the tail's deferral)."""
                seq = [("hh", 0, slice(0, 2)), ("x", 0, 0), ("x", 1, 1),
                       ("x", 2, 2), ("hh", 1, slice(2, 4)), ("x", 3, 3)]
                if upto is not None:
                    seq = seq[:upto]
                for i, (kind, idx, sl) in enumerate(seq):
                    start = (i == 0)
                    stop = (i == 5)
                    if kind == "hh":
                        nc.tensor.matmul(pp, aos[:, 0, sl, msl], wo_sb[:, 1, sl, esl],
                                         start=start, stop=stop, perf_mode=PM)
                    else:
                        nc.tensor.matmul(pp, aos[:, :, sl, msl], wo_sb[:, :, sl, esl],
                                         start=start, stop=stop, perf_mode=PM)
                    yield

            def out_rows(qc):
                return out_d[qc * QW:(qc + 1) * QW, :].rearrange(
                    "(m p) e -> p m e", p=P)

            def opj_gen(qc, aos):
                """Yield once per out-projection matmul for chunk qc
                (interleaved into the next chunk's attention loop). The four
                m-tiles of one ecn share a [P,4,512] eviction tile so each
                ecn issues a single out-DMA (HWDGE gen is 625ns per DMA)."""
                for ecn in range(NEC):
                    esl = slice(ecn * 512, (ecn + 1) * 512)
                    otc = otp.tile([P, 4, 512], bf16, tag="ot",
                                   name=f"ot_{qc}_{ecn}")
                    for m in range(QW // P):
                        msl = slice(m * P, (m + 1) * P)
                        pp = psP.tile([P, 512], f32, tag="pp",
                                      name=f"pp_{qc}_{ecn}_{m}")
                        gen = oproj_mms(pp[:, :], aos, msl, esl)
                        n = 0
                        for _ in gen:
                            n += 1
                            if n < 6:
                                yield
                        nc.vector.tensor_copy(otc[:, m, :], pp[:, :])
                        if m == QW // P - 1:
                            nc.sync.dma_start(out=out_rows(qc)[:, :, esl],
                                              in_=otc[:, :, :])
                        yield

            def opj_tail(qc, aos):
                """Un-interleaved tail out-projection: rotate pp tiles through
                all four (now idle) PSUM pools; defer each group's last two
                matmuls (hi*hi pair j=1 and cross fc=3, which need the last
                heads' aos whose normalization chains are still in flight)."""
                ppools = [(psP, "pp"), (psQ, "q"), (psO, "po"), (psS, "s")]
                finish = []

                otcs = {}

                def close_group(item):
                    pp, ecn, m, use_act = item
                    esl = slice(ecn * 512, (ecn + 1) * 512)
                    msl = slice(m * P, (m + 1) * P)
                    # remaining matmuls: hi*hi j=1, cross fc=3
                    nc.tensor.matmul(pp[:, :], aos[:, 0, 2:4, msl],
                                     wo_sb[:, 1, 2:4, esl],
                                     start=False, stop=False, perf_mode=PM)
                    nc.tensor.matmul(pp[:, :], aos[:, :, 3, msl],
                                     wo_sb[:, :, 3, esl],
                                     start=False, stop=True, perf_mode=PM)
                    g = ecn * (QW // P) + m
                    if ecn not in otcs:
                        otcs[ecn] = otp.tile([P, 4, 512], bf16, tag="ot",
                                             name=f"otT_{qc}_{ecn}")
                    otc = otcs[ecn]
                    if g % 2 == 0:
                        nc.scalar.copy(otc[:, m, :], pp[:, :])
                    else:
                        nc.vector.tensor_copy(otc[:, m, :], pp[:, :])
                    if m == 1:
                        nc.scalar.dma_start(out=out_rows(qc)[:, 0:2, esl],
                                            in_=otc[:, 0:2, :])
                    elif m == QW // P - 1:
                        nc.scalar.dma_start(out=out_rows(qc)[:, 2:4, esl],
                                            in_=otc[:, 2:4, :])

                for ecn in range(NEC):
                    esl = slice(ecn * 512, (ecn + 1) * 512)
                    for m in range(QW // P):
                        msl = slice(m * P, (m + 1) * P)
                        g = ecn * (QW // P) + m
                        pool, ptag = ppools[g % 4]
                        pp = pool.tile([P, 512], f32, tag=ptag,
                                       name=f"pp_{qc}_{ecn}_{m}")
                        # emit hi*hi j=0 and cross fc=0..2 (4 of 6); the rest
                        # (which need head 2-3 aos) go through close_group.
                        nc.tensor.matmul(pp[:, :], aos[:, 0, 0:2, msl],
                                         wo_sb[:, 1, 0:2, esl],
                                         start=True, stop=False, perf_mode=PM)
                        for fc in range(3):
                            nc.tensor.matmul(pp[:, :], aos[:, :, fc, msl],
                                             wo_sb[:, :, fc, esl],
                                             start=False, stop=False,
                                             perf_mode=PM)
                        finish.append((pp, ecn, m, g % 2 == 1))
                        if len(finish) > 6:
                            close_group(finish.pop(0))
                        if g >= 8 and finish:
                            close_group(finish.pop(0))
                        if g >= 12 and finish:
                            close_group(finish.pop(0))
                for item in finish:
                    close_group(item)

            def slot_list(opj, qpj):
                """192 lazy PE filler slots for one attention loop (3 per kv
                iteration): the first 16 draw from the out-projection stream
                only (lets the next hs block land before Q-projection starts),
                then the two streams alternate."""
                if opj is not None and qpj is not None:
                    order = []
                    o_left, q_left = 96, 96
                    for _ in range(8):
                        order.append(qpj)
                        q_left -= 1
                    for _ in range(8):
                        order.append(opj)
                        o_left -= 1
                    while o_left or q_left:
                        if q_left:
                            order.append(qpj)
                            q_left -= 1
                        if o_left:
                            order.append(opj)
                            o_left -= 1
                    return order
                # Single-stream chunks (first: no out-projection; last: no
                # next Q-projection): spread 96 fills over 192 slots.
                gen = opj if opj is not None else qpj
                if gen is None:
                    return []
                return [gen, None, gen] * 32 + [gen, None, None] * 32

            _pair = {}

            def make_scores_mm(qc, qT):
                def scores_mm(h, kv):
                    ps = psS.tile([P, QW], f32, tag="s", name=f"ps_{qc}_{h}_{kv}")
                    nc.tensor.matmul(
                        ps[:, :],
                        kT[h][:, kv * P:(kv + 1) * P],
                        qT[h][:, 0:QW],
                        start=True, stop=True,
                    )
                    if h >= H0DR:
                        if kv % 2 == 0:
                            _pair[h] = exl.tile([P, 2, QW], fp8, tag="exP",
                                                name=f"exP_{qc}_{h}_{kv}")
                        t = _pair[h]
                        nc.scalar.activation(t[:, kv % 2, :], ps[:, :], AF.Exp,
                                             bias=nbias[:, :], scale=SCALE)
                        return (t, kv % 2)
                    ex = exl.tile([P, QW], bf16, tag="ex",
                                  name=f"ex_{qc}_{h}_{kv}")
                    nc.scalar.activation(ex[:, :], ps[:, :], AF.Exp, scale=SCALE)
                    return ex
                return scores_mm

            def attention(qc, qT, slots, primed=None, next_scores=None):
                """Attention for chunk qc; three filler thunks per kv iter.
                primed: pre-emitted first-4 ex tiles (from the previous
                chunk's epilogue); next_scores: emit the next chunk's first 4
                scores+exp at our epilogue so its pipeline starts warm."""
                aos = aop.tile([P, 2, HPG, QW], mybir.dt.float8e4, tag="aos",
                               name=f"aos_{qc}")
                pending_norm = []
                si = iter(slots)

                def fill():
                    gen = next(si, None)
                    if gen is not None:
                        next(gen, None)

                scores_mm = make_scores_mm(qc, qT)
                po = exsumA = exsumB = None
                NIT = HPG * KVC
                exq = primed if primed is not None else \
                    [scores_mm(0, kv) for kv in range(4)]
                for hk in range(NIT):
                    h, kv = divmod(hk, KVC)
                    if kv == 0:
                        po = psO.tile([P, QW], f32, tag="po",
                                      name=f"po_{qc}_{h}")
                        exsumA = exs.tile([P, QW], f32, tag="exsumA",
                                          name=f"exsumA_{qc}_{h}")
                        exsumB = exs.tile([P, QW], f32, tag="exsumB",
                                          name=f"exsumB_{qc}_{h}")
                    ex = exq.pop(0)
                    if h >= H0DR:
                        exs_src = ex[0][:, ex[1], :]
                        if kv % 2 == 1:
                            t = ex[0]
                            dsl = slice((h - H0DR) * D, (h - H0DR + 1) * D)
                            nc.tensor.matmul(
                                po[:, :], vhi[:, kv - 1:kv + 1, dsl],
                                t[:, :, :], start=(kv == 1), stop=False,
                                perf_mode=PM)
                            nc.tensor.matmul(
                                po[:, :], vlo[:, kv - 1:kv + 1, dsl],
                                t[:, :, :], start=False, stop=(kv == KVC - 1),
                                perf_mode=PM)
                    else:
                        exs_src = ex[:, :]
                        nc.tensor.matmul(
                            po[:, :],
                            vS[kv][:, h * D:(h + 1) * D],
                            ex[:, :],
                            start=(kv == 0), stop=(kv == KVC - 1),
                        )
                    if kv == 0:
                        nc.vector.tensor_copy(exsumA[:, :], exs_src)
                    elif kv == 1:
                        nc.gpsimd.tensor_copy(exsumB[:, :], exs_src)
                    elif kv % 2 == 0:
                        nc.vector.tensor_add(exsumA[:, :], exsumA[:, :],
                                             exs_src)
                    else:
                        nc.gpsimd.tensor_add(exsumB[:, :], exsumB[:, :],
                                             exs_src)
                    fill()
                    if hk + 4 < NIT:
                        h2, kv2 = divmod(hk + 4, KVC)
                        exq.append(scores_mm(h2, kv2))
                    fill()
                    if kv == 1 and pending_norm:
                        pending_norm.pop(0)()
                    fill()
                    if kv == KVC - 1:
                        def make_norm(h=h, po=po, exsumA=exsumA,
                                      exsumB=exsumB):
                            def _norm():
                                sumb = rcp.tile([P, QW], f32, tag="sumb",
                                                name=f"sumb_{qc}_{h}")
                                recb = rcp.tile([P, QW], f32, tag="recb",
                                                name=f"recb_{qc}_{h}")
                                aot = rcp.tile([P, QW], f32, tag="aot",
                                               name=f"aot_{qc}_{h}")
                                nc.vector.tensor_add(
                                    exsumB[:, :], exsumB[:, :], exsumA[:, :])
                                nc.gpsimd.partition_all_reduce(
                                    sumb[:, :], exsumB[:, :], 128,
                                    ReduceOp.add)
                                nc.vector.reciprocal(recb[:, :], sumb[:, :])
                                nc.vector.tensor_mul(aot[:, :], po[:, :],
                                                     recb[:, :])
                                nc.vector.tensor_copy(aos[:, 0, h, :],
                                                      aot[:, :])
                                nc.vector.tensor_sub(aos[:, 1, h, :],
                                                     aot[:, :],
                                                     aos[:, 0, h, :])
                            return _norm
                        pending_norm.append(make_norm())

                next_primed = None
                if next_scores is not None:
                    next_primed = [next_scores(0, kv) for kv in range(4)]
                for fn in pending_norm:
                    fn()
                return aos, next_primed

            # Prologue: Q projection for chunk 0 runs standalone.
            qT_cur = new_qT(0)
            for _ in qproj_gen(0, hsb0, qT_cur):
                pass
            hsb = load_hsB(1)

            prev_aos = None
            prev_qc = None
            primed = None
            for qc in range(NQC):
                opj = opj_gen(prev_qc, prev_aos) if prev_aos is not None else None
                if qc + 1 < NQC:
                    qT_next = new_qT(qc + 1)
                    qpj = qproj_gen(qc + 1, hsb, qT_next)
                    nxt = make_scores_mm(qc + 1, qT_next)
                else:
                    qT_next, qpj, nxt = None, None, None
                aos, primed = attention(qc, qT_cur, slot_list(opj, qpj),
                                        primed=primed, next_scores=nxt)
                prev_aos, prev_qc = aos, qc
                qT_cur = qT_next
                if qc + 2 < NQC:
                    hsb = load_hsB(qc + 2)
            # tail: out-projection of the last chunk
            opj_tail(prev_qc, prev_aos)

    nc.compile()
    return nc


def get_nc():
    if "nc" not in _NC_CACHE:
        _NC_CACHE["nc"] = _build_nc()
    return _NC_CACHE["nc"]


def _rope_tables():
    inv_freq = (1.0 / (10000.0 ** (np.arange(0, D, 2, dtype=np.float32) / np.float32(D)))).astype(np.float32)
    t = np.arange(S, dtype=np.float32)
    freqs = t[:, None] * inv_freq[None, :]               # [S, 64]
    emb = np.concatenate([freqs, freqs], axis=1)         # [S, 128]
    cosT = np.ascontiguousarray(np.cos(emb).T.astype(np.float32))      # [128, S]
    sin = np.sin(emb).astype(np.float32)                 # [S, 128]
    sinTs = np.ascontiguousarray(
        np.concatenate([-sin[:, :64], sin[:, 64:]], axis=1).T.astype(np.float32)
    )                                                    # [128, S]
    return cosT, sinTs


def _bf16c(x):
    import ml_dtypes
    return np.ascontiguousarray(x).astype(ml_dtypes.bfloat16)


def _split8(x):
    """Split float32 array into (hi, lo) float8_e4m3 arrays."""
    import ml_dtypes
    F8 = ml_dtypes.float8_e4m3
    x = np.ascontiguousarray(x, dtype=np.float32)
    hi = x.astype(F8)
    lo = (x - hi.astype(np.float32)).astype(F8)
    return hi, lo


def _stack_hs(x):   # (hi, lo) order, [2, E, S]
    hi, lo = _split8(x * HS_SC)
    return np.ascontiguousarray(np.stack([hi, lo]))


def _stack_w(x):    # (lo, hi) order, [2, E, F]
    hi, lo = _split8(x * W_SC)
    return np.ascontiguousarray(np.stack([lo, hi]))


def _pack_hs(x):
    """[2, E, S] -> [NSB, P, 2*EC*SB]: dram[sb, p, (t, c, s)] = x[t, c*P+p, sb*SB+s]."""
    a = x.reshape(2, EC, P, NSB, SB)
    return np.ascontiguousarray(a.transpose(3, 2, 0, 1, 4).reshape(NSB, P, 2 * EC * SB))


def _pack_w(x, c):
    """[2, E_or_F, N] -> [P, 2*c*N]: dram[p, (t, cc, n)] = x[t, cc*P+p, n]."""
    n = x.shape[2]
    a = x.reshape(2, c, P, n)
    return np.ascontiguousarray(a.transpose(2, 0, 1, 3).reshape(P, 2 * c * n))


def make_in_maps(hidden_states, Wq, bq, Wk, bk, Wv, bv, Wo, bo):
    cosT, sinTs = _rope_tables()
    hsX = [_pack_hs(_stack_hs(np.asarray(hidden_states[b], dtype=np.float32).T))
           for b in range(B)]
    in_maps = []
    for c in range(8):
        b, g = divmod(c, G)
        gs = slice(g * F, (g + 1) * F)
        in_maps.append({
            "hsX": hsX[b],
            "wqX": _pack_w(_stack_w(np.asarray(Wq, dtype=np.float32)[gs, :].T), EC),
            "wkX": _pack_w(_stack_w(np.asarray(Wk, dtype=np.float32)[gs, :].T), EC),
            "wvX": _pack_w(_stack_w(np.asarray(Wv, dtype=np.float32)[gs, :].T), EC),
            "woX": _pack_w(_stack_w(np.asarray(Wo, dtype=np.float32)[:, gs].T), HPG),
            "bq2": np.ascontiguousarray(
                np.asarray(bq, dtype=np.float32)[gs].reshape(HPG, P).T),
            "bk2": np.ascontiguousarray(
                np.asarray(bk, dtype=np.float32)[gs].reshape(HPG, P).T),
            "cosT": _bf16c(cosT),
            "sinR": _bf16c(np.ascontiguousarray(np.roll(sinTs, 64, axis=0))),
        })
    return in_maps


def assemble_output(results, bv, Wo, bo):
    out = np.zeros((B, S, E), dtype=np.float32)
    for c in range(8):
        b = c // G
        out[b] += results[c]["out"].astype(np.float32)
    out *= O_DESC
    # v-bias folded out on device: softmax rows sum to 1, so the bv term is
    # the constant row bv @ Wo^T; add it with bo here.
    const_row = (np.asarray(bv, dtype=np.float32)
                 @ np.asarray(Wo, dtype=np.float32).T
                 + np.asarray(bo, dtype=np.float32))
    out += const_row[None, None, :]
    return out


def run_with_results(inputs, trace=False, **trace_kwargs):
    from concourse.bass_utils import run_bass_kernel_spmd
    nc = get_nc()
    in_maps = make_in_maps(**inputs)
    res = run_bass_kernel_spmd(nc, in_maps, list(range(8)), trace=trace, **trace_kwargs)
    out = assemble_output(res.results, inputs["bv"], inputs["Wo"], inputs["bo"])
    return out, res


def kernel(**inputs):
    out, _ = run_with_results(inputs)
    return out
